# revision 8
# baseline (speedup 1.0000x reference)
"""Bahdanau (additive) attention TRN2 Bass kernel.

reference:
    proj_in = einsum("bse,ea->bsa", inputs, W_in)      # [B,S,A]
    proj_q  = (query @ W_q)[:, None, :]                # [B,1,A]
    scores  = einsum("bsa,a->bs", tanh(proj_in+proj_q), w_att)
    weights = softmax(scores, axis=1)
    context = einsum("bs,bsa->ba", weights, proj_in)   # [B,A]

B,S,E,Q,A = 32,2048,1024,1024,512.

Sharding: data-parallel over batch. 8 cores x 4 batches each; weights
replicated. No collectives; host scatters inputs / gathers outputs.

Device algorithm per batch (bf16 matmuls, f32 PSUM accum):
  - proj_q for all local batches first, with the query free dim padded
    to 512 so these matmuls double as PE warmup (flips the HAM clock
    gate to 2.4 GHz before the main matmuls start).
  - X^T tiles [e,s] via DMA-transpose (bf16).
  - proj_in^T[a,s] = W_in^T X^T accumulated over e-chunks in PSUM.
    Each PSUM tile is drained twice: ACT tanh(+proj_q bias, per
    partition) -> t bf16, and DVE copy -> projT bf16 (kept for the
    context stage).
  - scores[1,s] = w_att^T t via PE matvec over the 4 a-chunks.
  - softmax without max subtraction (|scores| <= ~3 for this data):
    ACT Exp with accum_out produces the denominator in the same pass;
    exp is then normalized on ACT (Copy, scale=1/sum) and broadcast
    across partitions through a DRAM bounce.
  - context[a] = sum_s w[s] * projT[a,s] on DVE (mult + reduce), which
    is the final result; written out with 4 small per-a-tile DMAs.
"""

import sys

sys.path.insert(0, "/opt/trn_rl_repo")

import ml_dtypes
import numpy as np

import concourse.bass as bass
import concourse.tile as tile
from concourse import bacc, bass_utils, mybir

B, S, E, Q, A = 32, 2048, 1024, 1024, 512
NCORES = 8
BPC = B // NCORES  # batches per core
P = 128
EC = E // P  # 8 e-chunks
QC = Q // P  # 8 q-chunks
AT = A // P  # 4 a-tiles
SF = 512  # matmul moving free dim
SC = S // SF  # 4 s-chunks
QPAD = 512  # padded free dim for the proj_q warmup matmuls

BF = mybir.dt.bfloat16
F32 = mybir.dt.float32
TANH = mybir.ActivationFunctionType.Tanh
EXP = mybir.ActivationFunctionType.Exp
COPY = mybir.ActivationFunctionType.Copy


def build():
    nc = bacc.Bacc("TRN2", target_bir_lowering=False, debug=False)

    xT = nc.dram_tensor("xT", [BPC, E, S], BF, kind="ExternalInput")
    qT = nc.dram_tensor("qT", [Q, QPAD], BF, kind="ExternalInput")
    w_in = nc.dram_tensor("w_in", [E, A], BF, kind="ExternalInput")
    w_q = nc.dram_tensor("w_q", [Q, A], BF, kind="ExternalInput")
    w_att = nc.dram_tensor("w_att", [A], BF, kind="ExternalInput")
    out = nc.dram_tensor("out", [BPC, A], F32, kind="ExternalOutput")

    with tile.TileContext(nc) as tc:
        with (
            tc.tile_pool(name="const", bufs=1) as const,
            tc.tile_pool(name="xtp", bufs=2) as xtp,
            tc.tile_pool(name="ttp", bufs=2) as ttp,
            tc.tile_pool(name="small", bufs=3) as small,
            tc.tile_pool(name="mm_ps", bufs=4, space="PSUM") as mm_ps,
            tc.tile_pool(name="sc_ps", bufs=1, space="PSUM") as sc_ps,
            tc.tile_pool(name="dram", bufs=2, space="DRAM") as dram,
        ):
            # ---- constants (wq/qT first: the proj_q warmup depends on them)
            wq_sb = const.tile([P, QC, A], BF)
            nc.sync.dma_start(wq_sb, w_q.ap().rearrange("(qc p) a -> p qc a", p=P))
            qT_sb = const.tile([P, QC, QPAD], BF)
            nc.gpsimd.dma_start(qT_sb, qT.ap().rearrange("(qc p) b -> p qc b", p=P))
            watt_sb = const.tile([P, AT], BF)
            nc.gpsimd.dma_start(watt_sb, w_att.ap().rearrange("(at p) -> p at", p=P))
            w_sb = const.tile([P, EC, A], BF)
            nc.sync.dma_start(w_sb, w_in.ap().rearrange("(ec p) a -> p ec a", p=P))

            # ---- proj_q (padded to N=512: doubles as PE warmup for HAM)
            projq = []
            for at in range(AT):
                pq_ps = mm_ps.tile([P, SF], F32, name="mm_acc")
                for qc in range(QC):
                    nc.tensor.matmul(
                        pq_ps,
                        wq_sb[:, qc, at * P : (at + 1) * P],
                        qT_sb[:, qc, :],
                        start=(qc == 0),
                        stop=(qc == QC - 1),
                    )
                pq_sb = const.tile([P, BPC], F32, name=f"projq{at}")
                nc.scalar.copy(pq_sb, pq_ps[:, :BPC])
                projq.append(pq_sb)

            # software-pipelined: batch b's context stage is emitted during
            # iteration b+1 so the next batch's PSUM-draining casts stay
            # ahead of context work in the DVE queue.
            pending = None  # (b, projTall, wbc, rcp_bc)

            def emit_ctx(pend):
                pb, p_all, p_wbc, p_rcpbc = pend
                cscr_g = ttp.tile([P, S], BF, name="cscr_g", bufs=2)
                cscr_d = ttp.tile([P, S], BF, name="cscr_d", bufs=2)
                c = small.tile([P, AT], F32)
                # at 0/1 multiplied on GpSimd, at 2/3 on DVE; reduces on DVE
                for at in range(AT):
                    eng = nc.gpsimd if at < 2 else nc.vector
                    scr = cscr_g if at < 2 else cscr_d
                    eng.tensor_tensor(
                        out=scr,
                        in0=p_all[:, at * S : (at + 1) * S],
                        in1=p_wbc,
                        op=mybir.AluOpType.mult,
                    )
                    nc.vector.tensor_reduce(
                        c[:, at : at + 1],
                        scr,
                        axis=mybir.AxisListType.X,
                        op=mybir.AluOpType.add,
                    )
                nc.vector.tensor_scalar_mul(c, c, p_rcpbc)
                for at in range(AT):
                    nc.gpsimd.dma_start(
                        out.ap()[pb : pb + 1, at * P : (at + 1) * P],
                        c[:, at : at + 1],
                    )

            for b in range(BPC):
                # ---- X^T tiles (host pre-transposed): contiguous loads
                xts = []
                for ec in range(EC):
                    xt = xtp.tile([P, S], BF, name=f"xt{ec}")
                    nc.sync.dma_start(xt, xT.ap()[b, ec * P : (ec + 1) * P, :])
                    xts.append(xt)

                # ---- main matmul; drain PSUM twice (ACT tanh + DVE raw copy)
                ts_ = []
                projTall = ttp.tile([P, AT * S], BF, name="projTall", bufs=3)
                for at in range(AT):
                    t_sb = ttp.tile([P, S], BF, name=f"t{at}")
                    for sc in range(SC):
                        ps = mm_ps.tile([P, SF], F32, name="mm_acc")
                        for ec in range(EC):
                            nc.tensor.matmul(
                                ps,
                                w_sb[:, ec, at * P : (at + 1) * P],
                                xts[ec][:, sc * SF : (sc + 1) * SF],
                                start=(ec == 0),
                                stop=(ec == EC - 1),
                            )
                        nc.scalar.activation(
                            t_sb[:, sc * SF : (sc + 1) * SF],
                            ps,
                            TANH,
                            bias=projq[at][:, b : b + 1],
                        )
                        nc.vector.tensor_copy(
                            projTall[:, at * S + sc * SF : at * S + (sc + 1) * SF], ps
                        )
                    ts_.append(t_sb)

                # ---- scores: accumulate over a-chunks
                spss = [sc_ps.tile([1, SF], F32, name=f"sps{sc}") for sc in range(SC)]
                for at in range(AT):
                    for sc in range(SC):
                        nc.tensor.matmul(
                            spss[sc],
                            watt_sb[:, at : at + 1],
                            ts_[at][:, sc * SF : (sc + 1) * SF],
                            start=(at == 0),
                            stop=(at == AT - 1),
                        )

                # ---- exp + denominator + normalize
                exp_sb = small.tile([1, S], BF)
                sums = small.tile([1, SC], F32)
                for sc in range(SC):
                    nc.scalar.activation(
                        exp_sb[:, sc * SF : (sc + 1) * SF],
                        spss[sc],
                        EXP,
                        accum_out=sums[:, sc : sc + 1],
                    )
                tot = small.tile([1, 1], F32)
                nc.vector.tensor_reduce(
                    tot, sums, axis=mybir.AxisListType.X, op=mybir.AluOpType.add
                )
                rcp = small.tile([1, 1], F32)
                nc.vector.reciprocal(rcp, tot)

                # ---- broadcast raw exp and 1/sum across partitions
                exp_dram = dram.tile([1, S], BF)
                nc.gpsimd.dma_start(exp_dram, exp_sb)
                wbc = ttp.tile([P, S], BF, name="wbc")
                nc.gpsimd.dma_start(
                    wbc,
                    bass.AP(
                        tensor=exp_dram.tensor,
                        offset=exp_dram.offset,
                        ap=[[0, P], exp_dram.ap[-1]],
                    ),
                )
                rcp_dram = dram.tile([1, 1], F32, name="rcp_dram")
                nc.gpsimd.dma_start(rcp_dram, rcp)
                rcp_bc = small.tile([P, 1], F32, name="rcp_bc")
                nc.gpsimd.dma_start(
                    rcp_bc,
                    bass.AP(
                        tensor=rcp_dram.tensor,
                        offset=rcp_dram.offset,
                        ap=[[0, P], rcp_dram.ap[-1]],
                    ),
                )

                if pending is not None:
                    emit_ctx(pending)
                pending = (b, projTall, wbc, rcp_bc)

            emit_ctx(pending)

    nc.compile()
    return nc


_nc = None


def kernel(inputs, query, W_in, W_q, w_att):
    global _nc
    if _nc is None:
        _nc = build()

    bf = ml_dtypes.bfloat16
    x_bf = np.asarray(inputs).astype(bf)
    xT_bf = np.ascontiguousarray(x_bf.transpose(0, 2, 1))
    w_in_bf = np.ascontiguousarray(np.asarray(W_in).astype(bf))
    w_q_bf = np.ascontiguousarray(np.asarray(W_q).astype(bf))
    w_att_bf = np.ascontiguousarray(np.asarray(w_att).astype(bf))

    in_maps = []
    for c in range(NCORES):
        sl = slice(c * BPC, (c + 1) * BPC)
        qTp = np.zeros((Q, QPAD), dtype=bf)
        qTp[:, :BPC] = np.asarray(query[sl]).astype(bf).T
        in_maps.append(
            {
                "xT": np.ascontiguousarray(xT_bf[sl]),
                "qT": qTp,
                "w_in": w_in_bf,
                "w_q": w_q_bf,
                "w_att": w_att_bf,
            }
        )

    res = bass_utils.run_bass_kernel_spmd(_nc, in_maps, core_ids=list(range(NCORES)))
    return np.concatenate([r["out"] for r in res.results], axis=0)


if __name__ == "__main__":
    rng = np.random.default_rng(0)
    ins = {
        "inputs": rng.standard_normal((B, S, E), dtype=np.float32),
        "query": rng.standard_normal((B, Q), dtype=np.float32),
        "W_in": (rng.standard_normal((E, A), dtype=np.float32) / np.sqrt(E)).astype(
            np.float32
        ),
        "W_q": (rng.standard_normal((Q, A), dtype=np.float32) / np.sqrt(Q)).astype(
            np.float32
        ),
        "w_att": (rng.standard_normal((A,), dtype=np.float32) / np.sqrt(A)).astype(
            np.float32
        ),
    }
    got = kernel(**ins)
    print("out shape", got.shape, got.dtype)


# revision 9
# speedup vs baseline: 1.0327x; 1.0327x over previous
"""Bahdanau (additive) attention TRN2 Bass kernel.

reference:
    proj_in = einsum("bse,ea->bsa", inputs, W_in)      # [B,S,A]
    proj_q  = (query @ W_q)[:, None, :]                # [B,1,A]
    scores  = einsum("bsa,a->bs", tanh(proj_in+proj_q), w_att)
    weights = softmax(scores, axis=1)
    context = einsum("bs,bsa->ba", weights, proj_in)   # [B,A]

B,S,E,Q,A = 32,2048,1024,1024,512.

Sharding: data-parallel over batch. 8 cores x 4 batches each; weights
replicated. No collectives; host scatters inputs / gathers outputs.

Device algorithm per batch (bf16 matmuls, f32 PSUM accum):
  - proj_q for all local batches first, with the query free dim padded
    to 512 so these matmuls double as PE warmup (flips the HAM clock
    gate to 2.4 GHz before the main matmuls start).
  - X^T tiles [e,s] via DMA-transpose (bf16).
  - proj_in^T[a,s] = W_in^T X^T accumulated over e-chunks in PSUM.
    Each PSUM tile is drained twice: ACT tanh(+proj_q bias, per
    partition) -> t bf16, and DVE copy -> projT bf16 (kept for the
    context stage).
  - scores[1,s] = w_att^T t via PE matvec over the 4 a-chunks.
  - softmax without max subtraction (|scores| <= ~3 for this data):
    ACT Exp with accum_out produces the denominator in the same pass;
    exp is then normalized on ACT (Copy, scale=1/sum) and broadcast
    across partitions through a DRAM bounce.
  - context[a] = sum_s w[s] * projT[a,s] on DVE (mult + reduce), which
    is the final result; written out with 4 small per-a-tile DMAs.
"""

import sys

sys.path.insert(0, "/opt/trn_rl_repo")

import ml_dtypes
import numpy as np

import concourse.bass as bass
import concourse.tile as tile
from concourse import bacc, bass_utils, mybir

B, S, E, Q, A = 32, 2048, 1024, 1024, 512
NCORES = 8
BPC = B // NCORES  # batches per core
P = 128
EC = E // P  # 8 e-chunks
QC = Q // P  # 8 q-chunks
AT = A // P  # 4 a-tiles
SF = 512  # matmul moving free dim
SC = S // SF  # 4 s-chunks
QPAD = 512  # padded free dim for the proj_q warmup matmuls

BF = mybir.dt.bfloat16
F32 = mybir.dt.float32
TANH = mybir.ActivationFunctionType.Tanh
EXP = mybir.ActivationFunctionType.Exp
COPY = mybir.ActivationFunctionType.Copy


def build():
    nc = bacc.Bacc("TRN2", target_bir_lowering=False, debug=False)

    xT = nc.dram_tensor("xT", [BPC, E, S], BF, kind="ExternalInput")
    qT = nc.dram_tensor("qT", [Q, QPAD], BF, kind="ExternalInput")
    w_in = nc.dram_tensor("w_in", [E, A], BF, kind="ExternalInput")
    w_q = nc.dram_tensor("w_q", [Q, A], BF, kind="ExternalInput")
    w_att = nc.dram_tensor("w_att", [A], BF, kind="ExternalInput")
    out = nc.dram_tensor("out", [BPC, A], F32, kind="ExternalOutput")

    with tile.TileContext(nc) as tc:
        with (
            tc.tile_pool(name="const", bufs=1) as const,
            tc.tile_pool(name="xtp", bufs=2) as xtp,
            tc.tile_pool(name="ttp", bufs=2) as ttp,
            tc.tile_pool(name="small", bufs=3) as small,
            tc.tile_pool(name="mm_ps", bufs=4, space="PSUM") as mm_ps,
            tc.tile_pool(name="sc_ps", bufs=1, space="PSUM") as sc_ps,
            tc.tile_pool(name="dram", bufs=2, space="DRAM") as dram,
        ):
            # ---- constants (wq/qT first: the proj_q warmup depends on them)
            wq_sb = const.tile([P, QC, A], BF)
            nc.sync.dma_start(wq_sb, w_q.ap().rearrange("(qc p) a -> p qc a", p=P))
            qT_sb = const.tile([P, QC, QPAD], BF)
            nc.sync.dma_start(qT_sb, qT.ap().rearrange("(qc p) b -> p qc b", p=P))
            watt_sb = const.tile([P, AT], BF)
            nc.gpsimd.dma_start(watt_sb, w_att.ap().rearrange("(at p) -> p at", p=P))
            w_sb = const.tile([P, EC, A], BF)
            nc.sync.dma_start(w_sb, w_in.ap().rearrange("(ec p) a -> p ec a", p=P))

            # ---- proj_q (padded to N=512: doubles as PE warmup for HAM)
            projq = []
            for at in range(AT):
                pq_ps = mm_ps.tile([P, SF], F32, name="mm_acc")
                for qc in range(QC):
                    nc.tensor.matmul(
                        pq_ps,
                        wq_sb[:, qc, at * P : (at + 1) * P],
                        qT_sb[:, qc, :],
                        start=(qc == 0),
                        stop=(qc == QC - 1),
                    )
                pq_sb = const.tile([P, BPC], F32, name=f"projq{at}")
                nc.scalar.copy(pq_sb, pq_ps[:, :BPC])
                projq.append(pq_sb)

            # software-pipelined: batch b's context stage is emitted during
            # iteration b+1 so the next batch's PSUM-draining casts stay
            # ahead of context work in the DVE queue.
            pending = None  # (b, projTall, wbc, rcp_bc)

            def emit_ctx(pend):
                pb, p_all, p_wbc, p_rcpbc = pend
                cscr = ttp.tile([P, S], BF, name="cscr", bufs=1)
                c = small.tile([P, AT], F32)
                for at in range(AT):
                    nc.vector.tensor_tensor(
                        out=cscr,
                        in0=p_all[:, at * S : (at + 1) * S],
                        in1=p_wbc,
                        op=mybir.AluOpType.mult,
                    )
                    nc.vector.tensor_reduce(
                        c[:, at : at + 1],
                        cscr,
                        axis=mybir.AxisListType.X,
                        op=mybir.AluOpType.add,
                    )
                nc.vector.tensor_scalar_mul(c, c, p_rcpbc)
                for at in range(AT):
                    nc.gpsimd.dma_start(
                        out.ap()[pb : pb + 1, at * P : (at + 1) * P],
                        c[:, at : at + 1],
                    )

            for b in range(BPC):
                # ---- X^T tiles (host pre-transposed): contiguous loads
                xts = []
                for ec in range(EC):
                    xt = xtp.tile([P, S], BF, name=f"xt{ec}")
                    nc.sync.dma_start(xt, xT.ap()[b, ec * P : (ec + 1) * P, :])
                    xts.append(xt)

                # ---- main matmul; drain PSUM twice (ACT tanh + DVE raw copy)
                ts_ = []
                projTall = ttp.tile([P, AT * S], BF, name="projTall", bufs=3)
                for at in range(AT):
                    t_sb = ttp.tile([P, S], BF, name=f"t{at}")
                    for sc in range(SC):
                        ps = mm_ps.tile([P, SF], F32, name="mm_acc")
                        for ec in range(EC):
                            nc.tensor.matmul(
                                ps,
                                w_sb[:, ec, at * P : (at + 1) * P],
                                xts[ec][:, sc * SF : (sc + 1) * SF],
                                start=(ec == 0),
                                stop=(ec == EC - 1),
                            )
                        nc.scalar.activation(
                            t_sb[:, sc * SF : (sc + 1) * SF],
                            ps,
                            TANH,
                            bias=projq[at][:, b : b + 1],
                        )
                        nc.vector.tensor_copy(
                            projTall[:, at * S + sc * SF : at * S + (sc + 1) * SF], ps
                        )
                    ts_.append(t_sb)

                # ---- scores: accumulate over a-chunks
                spss = [sc_ps.tile([1, SF], F32, name=f"sps{sc}") for sc in range(SC)]
                for at in range(AT):
                    for sc in range(SC):
                        nc.tensor.matmul(
                            spss[sc],
                            watt_sb[:, at : at + 1],
                            ts_[at][:, sc * SF : (sc + 1) * SF],
                            start=(at == 0),
                            stop=(at == AT - 1),
                        )

                # ---- exp + denominator + normalize
                exp_sb = small.tile([1, S], BF)
                sums = small.tile([1, SC], F32)
                for sc in range(SC):
                    nc.scalar.activation(
                        exp_sb[:, sc * SF : (sc + 1) * SF],
                        spss[sc],
                        EXP,
                        accum_out=sums[:, sc : sc + 1],
                    )
                tot = small.tile([1, 1], F32)
                nc.vector.tensor_reduce(
                    tot, sums, axis=mybir.AxisListType.X, op=mybir.AluOpType.add
                )
                rcp = small.tile([1, 1], F32)
                nc.vector.reciprocal(rcp, tot)

                # ---- broadcast raw exp and 1/sum across partitions
                exp_dram = dram.tile([1, S], BF)
                nc.gpsimd.dma_start(exp_dram, exp_sb)
                wbc = ttp.tile([P, S], BF, name="wbc")
                nc.gpsimd.dma_start(
                    wbc,
                    bass.AP(
                        tensor=exp_dram.tensor,
                        offset=exp_dram.offset,
                        ap=[[0, P], exp_dram.ap[-1]],
                    ),
                )
                rcp_dram = dram.tile([1, 1], F32, name="rcp_dram")
                nc.gpsimd.dma_start(rcp_dram, rcp)
                rcp_bc = small.tile([P, 1], F32, name="rcp_bc")
                nc.gpsimd.dma_start(
                    rcp_bc,
                    bass.AP(
                        tensor=rcp_dram.tensor,
                        offset=rcp_dram.offset,
                        ap=[[0, P], rcp_dram.ap[-1]],
                    ),
                )

                if pending is not None:
                    emit_ctx(pending)
                pending = (b, projTall, wbc, rcp_bc)

            emit_ctx(pending)

    nc.compile()
    return nc


_nc = None


def kernel(inputs, query, W_in, W_q, w_att):
    global _nc
    if _nc is None:
        _nc = build()

    bf = ml_dtypes.bfloat16
    x_bf = np.asarray(inputs).astype(bf)
    xT_bf = np.ascontiguousarray(x_bf.transpose(0, 2, 1))
    w_in_bf = np.ascontiguousarray(np.asarray(W_in).astype(bf))
    w_q_bf = np.ascontiguousarray(np.asarray(W_q).astype(bf))
    w_att_bf = np.ascontiguousarray(np.asarray(w_att).astype(bf))

    in_maps = []
    for c in range(NCORES):
        sl = slice(c * BPC, (c + 1) * BPC)
        qTp = np.zeros((Q, QPAD), dtype=bf)
        qTp[:, :BPC] = np.asarray(query[sl]).astype(bf).T
        in_maps.append(
            {
                "xT": np.ascontiguousarray(xT_bf[sl]),
                "qT": qTp,
                "w_in": w_in_bf,
                "w_q": w_q_bf,
                "w_att": w_att_bf,
            }
        )

    res = bass_utils.run_bass_kernel_spmd(_nc, in_maps, core_ids=list(range(NCORES)))
    return np.concatenate([r["out"] for r in res.results], axis=0)


if __name__ == "__main__":
    rng = np.random.default_rng(0)
    ins = {
        "inputs": rng.standard_normal((B, S, E), dtype=np.float32),
        "query": rng.standard_normal((B, Q), dtype=np.float32),
        "W_in": (rng.standard_normal((E, A), dtype=np.float32) / np.sqrt(E)).astype(
            np.float32
        ),
        "W_q": (rng.standard_normal((Q, A), dtype=np.float32) / np.sqrt(Q)).astype(
            np.float32
        ),
        "w_att": (rng.standard_normal((A,), dtype=np.float32) / np.sqrt(A)).astype(
            np.float32
        ),
    }
    got = kernel(**ins)
    print("out shape", got.shape, got.dtype)
